# revision 1
# baseline (speedup 1.0000x reference)
"""Trainium2 Bass kernel for CrossAttention (B=2, N=M=2048, 16 heads x 64).

Sharding: batch x head-group parallel over 8 cores. Core c handles batch
c//4 and heads [4*(c%4), 4*(c%4)+4). Projection weights are column-split
(Wq/Wk/Wv) / row-split (Wo) per core; each core produces a partial
[2048, 1024] output which the host sums per batch (4 partials each).

Per-core device kernel (all matmuls f32r, full inputs pre-transposed on
host so the contraction dim lands on SBUF partitions):
  KT[i,m] = Wk_c^T ctx^T, QT[i,n] = Wq_c^T x^T   (k-outer accumulation
    into 8 psum banks so matmuls track the streaming input DMAs)
  V[m,i] = ctx Wv_c  (m on partitions; per-head ones column -> V_aug)
  attention per head-pair p, q-chunk: S^T[m,q] via K=64 row-packed
  matmul pairs (heads 2p/2p+1 on PE rows 0-63/64-127, concurrent),
  one Exp per m-tile (scale folded), O_aug = V_aug^T @ expS^T
  accumulated over m (row 64 = softmax denominators), normalize via
  DVE reciprocal + gpsimd partition_broadcast + DVE multiply,
  final = AO^T^T @ Wo_c.
"""

import numpy as np
from contextlib import ExitStack

import concourse.tile as tile
from concourse import bacc, mybir
from concourse.bass_utils import run_bass_kernel_spmd

B, N, M, C = 2, 2048, 2048, 1024
HEADS, D = 16, 64
HPC = 4            # heads per core
IC = HPC * D       # 256 inner dims per core
SCALE = D ** -0.5
NCORES = 8
KT_TILES = C // 128   # 8 contraction tiles for projections
f32 = mybir.dt.float32
f32r = mybir.dt.float32r

_CACHE = {}


def _body(nc, tc, ctx, xT, ctxT, wq, wk, wv, wo, out, taps=None, opt=None):
    opt = opt or {}
    O_BUFS = opt.get("o_bufs", 4)
    F_BUFS = opt.get("f_bufs", 2)
    S_BUFS = opt.get("s_bufs", 2)
    FUSE_FINAL = opt.get("fuse_final", False)
    P = 128
    MT = M // P   # 16 m tiles

    const = ctx.enter_context(tc.tile_pool(name="const", bufs=1))
    proj_out = ctx.enter_context(tc.tile_pool(name="proj_out", bufs=1))

    # persistent small constants / output-projection weight
    wo_sb = const.tile([P, 2, C], f32r, tag="wo")
    ones_sb = const.tile([P, 1], f32, tag="ones")
    nc.vector.memset(ones_sb[:], 1.0)

    kt_sb = [proj_out.tile([P, M], f32r, tag=f"kt{j}", name=f"kt{j}") for j in range(2)]
    qt_sb = [proj_out.tile([P, N], f32r, tag=f"qt{j}", name=f"qt{j}") for j in range(2)]
    v_sb = proj_out.tile([P, MT, HPC, D + 1], f32r, tag="v")

    nc.vector.tensor_copy(
        v_sb[:, :, :, D:D + 1],
        ones_sb[:, 0:1].to_broadcast((P, MT, HPC, 1)),
    )

    # ---- projections: KT, QT (k-outer, 8 psum banks); V fused into attention
    vw_pool = ctx.enter_context(tc.tile_pool(name="vw", bufs=1))
    ctx_pool = ctx.enter_context(tc.tile_pool(name="ctxp", bufs=1))
    with (
        tc.tile_pool(name="wqkv", bufs=1) as wpool,
        tc.tile_pool(name="xk", bufs=7) as xkp,
        tc.tile_pool(name="pp8", bufs=8, space="PSUM") as pps,
    ):
        wq_sb = wpool.tile([P, KT_TILES, IC], f32r, tag="wq")
        wk_sb = wpool.tile([P, KT_TILES, IC], f32r, tag="wk")
        wv_sb = vw_pool.tile([P, KT_TILES, IC], f32r, tag="wv")
        ctx_sb = ctx_pool.tile([P, KT_TILES, M], f32r, tag="ctx")
        # 3 DMA lanes: SP: ctx0-5, x5-7 ; Pool: ctx6-7, x3-4 ;
        # ACT: wk, wq, x0-2, wv, wo
        for k in range(5):
            nc.sync.dma_start(ctx_sb[:, k, :], ctxT[k * P:(k + 1) * P, :])
        for k in range(5, KT_TILES):
            nc.gpsimd.dma_start(ctx_sb[:, k, :], ctxT[k * P:(k + 1) * P, :])
        for k in range(KT_TILES):
            nc.scalar.dma_start(wk_sb[:, k, :], wk[k * P:(k + 1) * P, :])
        for k in range(KT_TILES):
            nc.scalar.dma_start(wq_sb[:, k, :], wq[k * P:(k + 1) * P, :])
        x_tiles = {}
        for k in range(7):
            x_k = xkp.tile([P, N], f32r, tag="xk", name=f"xk{k}")
            eng = nc.scalar if k < 3 else nc.gpsimd
            eng.dma_start(x_k[:], xT[k * P:(k + 1) * P, :])
            x_tiles[k] = x_k
        for k in range(KT_TILES):
            nc.scalar.dma_start(wv_sb[:, k, :], wv[k * P:(k + 1) * P, :])
        for j in range(2):
            nc.scalar.dma_start(wo_sb[:, j, :], wo[j * P:(j + 1) * P, :])

        # KT: 8 outputs (j, qc) accumulate k-outer, tracking ctx DMA arrival
        kpsum = [pps.tile([P, 512], f32, tag="pp", name=f"kp{i}") for i in range(8)]
        for k in range(KT_TILES):
            for j in range(2):
                for qc in range(4):
                    nc.tensor.matmul(
                        kpsum[j * 4 + qc][:],
                        wk_sb[:, k, j * P:(j + 1) * P],
                        ctx_sb[:, k, qc * 512:(qc + 1) * 512],
                        start=(k == 0), stop=(k == KT_TILES - 1),
                    )
        for j in range(2):
            for qc in range(4):
                eng = nc.vector.tensor_copy if (j * 4 + qc) % 2 == 0 else nc.scalar.copy
                eng(kt_sb[j][:, qc * 512:(qc + 1) * 512], kpsum[j * 4 + qc][:])

        # QT: same shape, x streamed per k-tile on the ACT queue
        qpsum = [pps.tile([P, 512], f32, tag="pp", name=f"qp{i}") for i in range(8)]
        for k in range(KT_TILES):
            if k in x_tiles:
                x_k = x_tiles[k]
            else:
                x_k = xkp.tile([P, N], f32r, tag="xk", name=f"xk{k}")
                nc.sync.dma_start(x_k[:], xT[k * P:(k + 1) * P, :])
            for j in range(2):
                for qc in range(4):
                    nc.tensor.matmul(
                        qpsum[j * 4 + qc][:],
                        wq_sb[:, k, j * P:(j + 1) * P],
                        x_k[:, qc * 512:(qc + 1) * 512],
                        start=(k == 0), stop=(k == KT_TILES - 1),
                    )
        for j in range(2):
            for qc in range(4):
                eng = nc.vector.tensor_copy if (j * 4 + qc) % 2 == 0 else nc.scalar.copy
                eng(qt_sb[j][:, qc * 512:(qc + 1) * 512], qpsum[j * 4 + qc][:])
    if taps is not None:
        for j in range(2):
            nc.sync.dma_start(taps["d_qt"][j], qt_sb[j][:])
            nc.sync.dma_start(taps["d_kt"][j], kt_sb[j][:])
        nc.sync.dma_start(taps["d_v"][:], v_sb[:])

    # ---- attention ----
    ao_pool = ctx.enter_context(tc.tile_pool(name="ao", bufs=1))
    ao_sb = [ao_pool.tile([P, N], f32r, tag=f"ao{j}", name=f"ao{j}") for j in range(2)]
    es_pool = ctx.enter_context(tc.tile_pool(name="es", bufs=4))
    small = ctx.enter_context(tc.tile_pool(name="small", bufs=2))
    QC = 512
    out_pool = ctx.enter_context(tc.tile_pool(name="outp", bufs=3))
    with ExitStack() as attn_ctx:
        sps = attn_ctx.enter_context(tc.tile_pool(name="s_ps", bufs=S_BUFS, space="PSUM"))
        ops = attn_ctx.enter_context(tc.tile_pool(name="o_ps", bufs=O_BUFS, space="PSUM"))
        if opt.get("share_of"):
            fps = ops
        elif FUSE_FINAL:
            fps = attn_ctx.enter_context(tc.tile_pool(name="f_ps", bufs=F_BUFS, space="PSUM"))
        else:
            fps = None
        for qc in range(N // QC):
            q0 = qc * QC
            for p in range(2):       # head pair (local heads 2p, 2p+1)
                o_ts = [ops.tile([P, QC], f32, tag="o", name=f"o{i}") for i in range(2)]
                for mt in range(MT):
                    s_t = sps.tile([P, 2 * QC], f32, tag="s")
                    for hh in range(2):  # row-packed pair, concurrent on PE
                        pb = hh * 64
                        nc.tensor.matmul(
                            s_t[:, hh * QC:(hh + 1) * QC],
                            kt_sb[p][pb:pb + 64, mt * P:(mt + 1) * P],
                            qt_sb[p][pb:pb + 64, q0:q0 + QC],
                            start=True, stop=True,
                        )
                    es = es_pool.tile([P, 2 * QC], f32r, tag="es")
                    nc.scalar.activation(
                        es[:], s_t[:],
                        mybir.ActivationFunctionType.Exp, scale=SCALE,
                    )
                    if taps is not None and p == 0 and qc == 0 and mt == 0:
                        nc.sync.dma_start(taps["d_es"][:], es[:])
                    if qc == 0 and p == 0:
                        vt = ops.tile([P, IC], f32, tag="o", name=f"vt{mt}")
                        for k in range(KT_TILES):
                            nc.tensor.matmul(
                                vt[:],
                                ctx_sb[:, k, mt * P:(mt + 1) * P],
                                wv_sb[:, k, :],
                                start=(k == 0), stop=(k == KT_TILES - 1),
                            )
                        nc.vector.tensor_copy(
                            v_sb[:, mt, :, 0:D],
                            vt[:].rearrange("p (h d) -> p h d", d=D),
                        )
                    for hh in range(2):
                        h = 2 * p + hh
                        nc.tensor.matmul(
                            o_ts[hh][0:D + 1, :],
                            v_sb[:, mt, h, :],
                            es[:, hh * QC:(hh + 1) * QC],
                            start=(mt == 0), stop=(mt == MT - 1),
                        )
                for hh in range(2):
                    o_t = o_ts[hh]
                    r_sb = small.tile([P, QC], f32, tag="r")
                    nc.vector.reciprocal(r_sb[64:65, :], o_t[D:D + 1, :])
                    nc.gpsimd.dma_start(r_sb[0:1, :], r_sb[64:65, :])
                    rb_sb = small.tile([P, QC], f32, tag="rb")
                    nc.gpsimd.partition_broadcast(rb_sb[0:D, :], r_sb[0:1, :])
                    if taps is not None and p == 0 and qc == 0 and hh == 0:
                        ot_sb = small.tile([P, QC], f32, tag="ot_dbg")
                        nc.vector.tensor_copy(ot_sb[0:D + 1, :], o_t[0:D + 1, :])
                        nc.sync.dma_start(taps["d_o"][:], ot_sb[:])
                        nc.sync.dma_start(taps["d_r"][:], r_sb[:])
                        nc.sync.dma_start(taps["d_rb"][:], rb_sb[:])
                    if hh == 0:
                        nc.vector.tensor_mul(
                            ao_sb[p][0:D, q0:q0 + QC], o_t[0:D, :], rb_sb[0:D, :]
                        )
                    else:
                        ao_tmp = small.tile([P, QC], f32r, tag="aot")
                        nc.vector.tensor_mul(
                            ao_tmp[0:D, :], o_t[0:D, :], rb_sb[0:D, :]
                        )
                        nc.gpsimd.dma_start(
                            ao_sb[p][64:128, q0:q0 + QC], ao_tmp[0:D, :]
                        )

            # fused output projection for this q-chunk's 4 n-tiles
            for nt in (range(qc * 4, qc * 4 + 4) if FUSE_FINAL else []):
                for ec in range(C // 512):
                    ft = fps.tile([P, 512], f32, tag=("o" if opt.get("share_of") else "f"), name="ft")
                    for j in range(2):
                        nc.tensor.matmul(
                            ft[:],
                            ao_sb[j][:, nt * P:(nt + 1) * P],
                            wo_sb[:, j, ec * 512:(ec + 1) * 512],
                            start=(j == 0), stop=(j == 1),
                        )
                    o_sb = out_pool.tile([P, 512], f32, tag="ot")
                    nc.vector.tensor_copy(o_sb[:], ft[:])
                    nc.sync.dma_start(
                        out[nt * P:(nt + 1) * P, ec * 512:(ec + 1) * 512], o_sb[:]
                    )

    if not FUSE_FINAL:
        with tc.tile_pool(name="f2_ps", bufs=2, space="PSUM") as fps2:
            for nt in range(N // P):
                ft = fps2.tile([P, C], f32, tag="f")
                for ec in range(C // 512):
                    for j in range(2):
                        nc.tensor.matmul(
                            ft[:, ec * 512:(ec + 1) * 512],
                            ao_sb[j][:, nt * P:(nt + 1) * P],
                            wo_sb[:, j, ec * 512:(ec + 1) * 512],
                            start=(j == 0), stop=(j == 1),
                        )
                o_sb = out_pool.tile([P, C], f32, tag="ot")
                nc.vector.tensor_copy(o_sb[:], ft[:])
                eng = nc.sync if nt % 2 == 0 else nc.scalar
                eng.dma_start(out[nt * P:(nt + 1) * P, :], o_sb[:])

    if taps is not None:
        for j in range(2):
            nc.sync.dma_start(taps["d_ao"][j], ao_sb[j][:])


def _build(reps=1, opt=None):
    key = (reps, tuple(sorted((opt or {}).items())))
    if key in _CACHE:
        return _CACHE[key]
    nc = bacc.Bacc("TRN2", target_bir_lowering=False, debug=False)
    xT = nc.dram_tensor("xT", [C, N], f32r, kind="ExternalInput")
    ctxT = nc.dram_tensor("ctxT", [C, M], f32r, kind="ExternalInput")
    wq = nc.dram_tensor("wq", [C, IC], f32r, kind="ExternalInput")
    wk = nc.dram_tensor("wk", [C, IC], f32r, kind="ExternalInput")
    wv = nc.dram_tensor("wv", [C, IC], f32r, kind="ExternalInput")
    wo = nc.dram_tensor("wo", [IC, C], f32r, kind="ExternalInput")
    out = nc.dram_tensor("out", [N, C], f32, kind="ExternalOutput")
    with tile.TileContext(nc) as tc:
        for _ in range(reps):
            with ExitStack() as ctx:
                _body(nc, tc, ctx, xT, ctxT, wq, wk, wv, wo, out, opt=opt)
    nc.compile()
    _CACHE[key] = nc
    return nc


def _shard_inputs(x, context, Wq, Wk, Wv, Wo):
    in_maps = []
    for c in range(NCORES):
        b, g = divmod(c, NCORES // B)
        cols = slice(g * IC, (g + 1) * IC)
        in_maps.append({
            "xT": np.ascontiguousarray(x[b].T),
            "ctxT": np.ascontiguousarray(context[b].T),
            "wq": np.ascontiguousarray(Wq[:, cols]),
            "wk": np.ascontiguousarray(Wk[:, cols]),
            "wv": np.ascontiguousarray(Wv[:, cols]),
            "wo": np.ascontiguousarray(Wo[cols, :]),
        })
    return in_maps


def kernel(x, context, Wq, Wk, Wv, Wo, reps=1):
    x = np.asarray(x, dtype=np.float32)
    context = np.asarray(context, dtype=np.float32)
    Wq, Wk, Wv, Wo = (np.asarray(w, dtype=np.float32) for w in (Wq, Wk, Wv, Wo))
    nc = _build(reps)
    in_maps = _shard_inputs(x, context, Wq, Wk, Wv, Wo)
    res = run_bass_kernel_spmd(nc, in_maps, core_ids=list(range(NCORES)))
    gpb = NCORES // B
    out = np.zeros((B, N, C), dtype=np.float32)
    for c in range(NCORES):
        out[c // gpb] += res.results[c]["out"]
    return out

